# Initial kernel scaffold
#
"""Trainium2 Bass kernel for the CGKGR GNN message-passing model.

Contract: kernel(**inputs) takes FULL (unsharded) numpy inputs as produced by
the reference setup_inputs(), returns (scores, probs) — the same structure the
reference returns. Internally: data-parallel over the batch dim across 8
NeuronCores (32 rows each); embedding tables / adjacency / weights replicated.

Algorithm notes (per core, B_L=32, S=8, D=64, NL=2, NR=32):
  * All embedding/adjacency lookups are GPSIMD indirect DMAs in a "packed"
    layout: an index tile [P, k] gathers rows into [P, k*64] so each partition
    holds whole attention groups; chained gathers compose with no partition
    shuffles.
  * Attention scores never materialize messages: <u, X@W_r> == <X, W_r@u>,
    with Wu[b,r] = W_r @ new_u[b] built once on device (33 small matmuls),
    bounced to DRAM, and gathered per edge by index b*33+rel.
  * Relation-weighted aggregation sum_s att*X@W[rel] is computed as
    sum_r Z[g,r,:] @ (W_r@W1) where Z is built by one-hot matmuls
    (onehot[e, g_local*33+rel] built from an iota constant + is_equal) and the
    33 products accumulate into one PSUM tile with W_r@W1 pre-composed.
"""

import sys

try:
    import concourse  # noqa: F401
except ImportError:  # pragma: no cover
    sys.path.insert(0, "/opt/trn_rl_repo")

import numpy as np

from concourse import bass, bacc, mybir
import concourse.tile as tile
from concourse.bass_utils import run_bass_kernel_spmd

f32 = mybir.dt.float32
i32 = mybir.dt.int32
AF = mybir.ActivationFunctionType
OP = mybir.AluOpType
AX = mybir.AxisListType

# ---- problem config (hardcoded; must match the reference) ----
B = 256
BL = 32          # batch rows per core
NCORES = 8
S = 8
D = 64
NL = 2
NU = 50000
NI = 20000
NE = 200000
NR = 32
NW = NR + 1      # W_R slots (incl. user-item relation at slot NR)
P = 128

E1 = BL * S * S  # 2048 hop-1 edges per core
G1 = BL * S      # 256  hop-1 groups per core
E0 = BL * S      # 256  hop-0 edges per core
T1 = E1 // P     # 16 flat tiles (hop-1)
T0 = E0 // P     # 2  flat tiles (hop-0)
OH = 16 * NW     # 528 one-hot columns per flat tile (16 groups x 33 rels)

_CACHE = {}


def _host_consts():
    p = np.arange(P)
    j16 = np.arange(16)
    consts = {
        "ident": np.eye(P, dtype=np.float32),
        "iotaJ": np.broadcast_to(np.arange(OH, dtype=np.float32), (P, OH)).copy(),
        # hop-1 packed (p, j): edge e = p*16+j
        "b33p1": (((p[:, None] * 16 + j16[None, :]) // (S * S)) * NW).astype(np.float32),
        "g33p1": ((2 * (p[:, None] % 8) + j16[None, :] // 8) * NW).astype(np.float32),
        # hop-0 packed (p, j): edge e = p*8+j, p in [0,32)
        "b33c0": (np.arange(BL)[:, None] * NW).astype(np.float32),
        "g33c0": ((np.arange(BL)[:, None] % 16) * NW).astype(np.float32),
        "ones64": np.ones((D, 1), dtype=np.float32),
    }
    return consts


def _build_nc():
    nc = bacc.Bacc(None, target_bir_lowering=False, debug=False, num_devices=NCORES)

    # ---- I/O ----
    user_idx = nc.declare_dram_parameter("user_idx", [BL], i32, isOutput=False)
    item_idx = nc.declare_dram_parameter("item_idx", [BL], i32, isOutput=False)
    adj_u2i = nc.declare_dram_parameter("adj_u2i", [NU, S], i32, isOutput=False)
    adj_i2u = nc.declare_dram_parameter("adj_i2u", [NI, S], i32, isOutput=False)
    adj_e2e = nc.declare_dram_parameter("adj_e2e", [NE, S], i32, isOutput=False)
    adj_rel = nc.declare_dram_parameter("adj_rel", [NE, S], i32, isOutput=False)
    user_emb = nc.declare_dram_parameter("user_emb", [NU, D], f32, isOutput=False)
    entity_emb = nc.declare_dram_parameter("entity_emb", [NE, D], f32, isOutput=False)
    # W_R pre-transposed on host: WRT[d_in, r*64+d_out_pre]... layout [64, 33*64]
    # with WRT[:, r*64:(r+1)*64] == W_R[r].T
    w_rt = nc.declare_dram_parameter("w_rt", [D, NW * D], f32, isOutput=False)
    agg_wu = nc.declare_dram_parameter("agg_wu", [D, D], f32, isOutput=False)
    agg_bu = nc.declare_dram_parameter("agg_bu", [D], f32, isOutput=False)
    # ent_W1 re-laid on host: [d_in, layer*64+d_out]
    w1both = nc.declare_dram_parameter("w1both", [D, NL * D], f32, isOutput=False)
    ent_b1 = nc.declare_dram_parameter("ent_b1", [NL, D], f32, isOutput=False)
    ent_w2 = nc.declare_dram_parameter("ent_w2", [D, D], f32, isOutput=False)
    ent_b2 = nc.declare_dram_parameter("ent_b2", [D], f32, isOutput=False)

    ident_c = nc.declare_dram_parameter("ident", [P, P], f32, isOutput=False)
    iotaJ_c = nc.declare_dram_parameter("iotaJ", [P, OH], f32, isOutput=False)
    b33p1_c = nc.declare_dram_parameter("b33p1", [P, 16], f32, isOutput=False)
    g33p1_c = nc.declare_dram_parameter("g33p1", [P, 16], f32, isOutput=False)
    b33c0_c = nc.declare_dram_parameter("b33c0", [BL, 1], f32, isOutput=False)
    g33c0_c = nc.declare_dram_parameter("g33c0", [BL, 1], f32, isOutput=False)
    ones64_c = nc.declare_dram_parameter("ones64", [D, 1], f32, isOutput=False)

    scores_out = nc.declare_dram_parameter("scores_out", [BL], f32, isOutput=True)
    probs_out = nc.declare_dram_parameter("probs_out", [BL], f32, isOutput=True)

    WUI = slice(NR * D, NW * D)  # W_ui^T slice of w_rt

    with tile.TileContext(nc) as tc:
        with (
            tc.tile_pool(name="const", bufs=1) as cp,
            tc.tile_pool(name="wpool", bufs=1) as wp,
            tc.tile_pool(name="sb", bufs=2) as sb,
            tc.tile_pool(name="big", bufs=1) as bigp,
            tc.tile_pool(name="ps", bufs=2, space="PSUM") as ps,
            tc.tile_pool(name="psacc", bufs=2, space="PSUM") as psa,
            tc.tile_pool(name="dram", bufs=1, space="DRAM") as dp,
        ):
            # ---------------- constants / weights into SBUF ----------------
            ident = cp.tile([P, P], f32)
            nc.sync.dma_start(out=ident[:], in_=ident_c[:])
            iotaJ = cp.tile([P, OH], f32)
            nc.sync.dma_start(out=iotaJ[:], in_=iotaJ_c[:])
            b33p1 = cp.tile([P, 16], f32)
            nc.sync.dma_start(out=b33p1[:], in_=b33p1_c[:])
            g33p1 = cp.tile([P, 16], f32)
            nc.sync.dma_start(out=g33p1[:], in_=g33p1_c[:])
            b33c0 = cp.tile([BL, 1], f32)
            nc.sync.dma_start(out=b33c0[:], in_=b33c0_c[:])
            g33c0 = cp.tile([BL, 1], f32)
            nc.sync.dma_start(out=g33c0[:], in_=g33c0_c[:])
            ones64 = cp.tile([D, 1], f32)
            nc.sync.dma_start(out=ones64[:], in_=ones64_c[:])

            wrt = wp.tile([D, NW * D], f32)
            nc.sync.dma_start(out=wrt[:], in_=w_rt[:])
            aggW = wp.tile([D, D], f32)
            nc.sync.dma_start(out=aggW[:], in_=agg_wu[:])
            w1b = wp.tile([D, NL * D], f32)
            nc.sync.dma_start(out=w1b[:], in_=w1both[:])
            w2 = wp.tile([D, D], f32)
            nc.sync.dma_start(out=w2[:], in_=ent_w2[:])
            aggb = wp.tile([D, 1], f32)
            nc.sync.dma_start(out=aggb[:], in_=agg_bu[:, None])
            b1c0 = wp.tile([D, 1], f32)
            nc.sync.dma_start(out=b1c0[:], in_=ent_b1[0, :, None])
            b1c1 = wp.tile([D, 1], f32)
            nc.sync.dma_start(out=b1c1[:], in_=ent_b1[1, :, None])
            b2c = wp.tile([D, 1], f32)
            nc.sync.dma_start(out=b2c[:], in_=ent_b2[:, None])
            bL1 = wp.tile([D, 1], f32)
            nc.vector.tensor_tensor(out=bL1[:], in0=b1c1[:], in1=b2c[:], op=OP.add)

            # pre-composed Wcomb[r, layer] = W_r @ ent_W1[layer]; [64, (r, layer, 64)]
            wcomb = wp.tile([D, NW * NL * D], f32)
            for r in range(NW):
                pc = ps.tile([D, NL * D], f32, space="PSUM", tag="wc_ps")
                nc.tensor.matmul(out=pc[:], lhsT=wrt[:, r * D:(r + 1) * D],
                                 rhs=w1b[:], start=True, stop=True)
                nc.vector.tensor_copy(out=wcomb[:, r * NL * D:(r + 1) * NL * D], in_=pc[:])

            def wc(r, layer):
                off = r * NL * D + layer * D
                return wcomb[:, off:off + D]

            # WcombA = W_ui @ agg_Wu ; Wcomb2 = W_ui @ ent_W2
            wcA = wp.tile([D, D], f32)
            pcA = ps.tile([D, D], f32, space="PSUM", tag="wc_ps")
            nc.tensor.matmul(out=pcA[:], lhsT=wrt[:, WUI], rhs=aggW[:], start=True, stop=True)
            nc.vector.tensor_copy(out=wcA[:], in_=pcA[:])
            wc2 = wp.tile([D, D], f32)
            pc2 = ps.tile([D, D], f32, space="PSUM", tag="wc_ps")
            nc.tensor.matmul(out=pc2[:], lhsT=wrt[:, WUI], rhs=w2[:], start=True, stop=True)
            nc.vector.tensor_copy(out=wc2[:], in_=pc2[:])

            # ---------------- DRAM scratch ----------------
            e1dram = dp.tile([G1, 1], i32)
            wu_dram = dp.tile([BL * NW, D], f32)
            v1_dram = dp.tile([E1, D], f32)
            v0_dram = dp.tile([E0, D], f32)
            oi1_dram = dp.tile([E1, 1], f32)
            oi0_dram = dp.tile([E0, 1], f32)
            h1_dram = dp.tile([G1, D], f32)
            aL1_dram = dp.tile([E0, 1], f32)

            def gather(out_ap, table_ap, idx_ap):
                nc.gpsimd.indirect_dma_start(
                    out=out_ap, out_offset=None, in_=table_ap,
                    in_offset=bass.IndirectOffsetOnAxis(ap=idx_ap, axis=0))

            def transpose_ps(in_sb, n_part, n_free, tag):
                """PE transpose: in_sb [n_part, n_free] -> psum [n_free, n_part]."""
                t = ps.tile([n_free, n_part], f32, space="PSUM", tag=tag)
                nc.tensor.transpose(out=t[:], in_=in_sb, identity=ident[:n_part, :n_part])
                return t

            # ---------------- index / embedding gather chains ----------------
            uidx = sb.tile([BL, 1], i32, tag="uidx")
            nc.sync.dma_start(out=uidx[:], in_=user_idx[:, None])
            iidx = sb.tile([BL, 1], i32, tag="iidx")
            nc.sync.dma_start(out=iidx[:], in_=item_idx[:, None])

            adjU = sb.tile([BL, S], i32, tag="adjU")
            gather(adjU[:], adj_u2i[:], uidx[:, :1])
            x0u = bigp.tile([BL, S * D], f32)            # item neighbors of users
            gather(x0u[:], entity_emb[:], adjU[:])

            ur = sb.tile([BL, D], f32, tag="ur")         # u rows
            gather(ur[:], user_emb[:], uidx[:, :1])
            uT_ps = transpose_ps(ur[:], BL, D, "tp64")
            uT = sb.tile([D, BL], f32, tag="uT")
            nc.vector.tensor_copy(out=uT[:], in_=uT_ps[:])

            adjI = sb.tile([BL, S], i32, tag="adjI")
            gather(adjI[:], adj_i2u[:], iidx[:, :1])
            xu = bigp.tile([BL, S * D], f32)             # user neighbors of items
            gather(xu[:], user_emb[:], adjI[:])

            ier = sb.tile([BL, D], f32, tag="ier")       # item entity rows
            gather(ier[:], entity_emb[:], iidx[:, :1])
            ieT_ps = transpose_ps(ier[:], BL, D, "tp64")
            ieT = sb.tile([D, BL], f32, tag="ieT")
            nc.vector.tensor_copy(out=ieT[:], in_=ieT_ps[:])

            ents1 = sb.tile([BL, S], i32, tag="ents1")
            gather(ents1[:], adj_e2e[:], iidx[:, :1])
            rel0 = sb.tile([BL, S], i32, tag="rel0")
            gather(rel0[:], adj_rel[:], iidx[:, :1])

            # bounce ents1 -> DRAM; reload packed-2 and flat column layouts
            nc.sync.dma_start(
                out=e1dram[:].rearrange("(p j) o -> p (j o)", p=BL),
                in_=ents1[:])
            e1col = sb.tile([P, 2], i32, tag="e1col")
            nc.sync.dma_start(out=e1col[:], in_=e1dram[:].rearrange("(p j) o -> p (j o)", p=P))
            e1f0 = sb.tile([P, 1], i32, tag="e1f0")
            nc.sync.dma_start(out=e1f0[:], in_=e1dram[0:P, :])
            e1f1 = sb.tile([P, 1], i32, tag="e1f1")
            nc.sync.dma_start(out=e1f1[:], in_=e1dram[P:2 * P, :])

            ents2 = sb.tile([P, 16], i32, tag="ents2")
            gather(ents2[:], adj_e2e[:], e1col[:])
            rel1 = sb.tile([P, 16], i32, tag="rel1")
            gather(rel1[:], adj_rel[:], e1col[:])
            x1 = bigp.tile([P, 16 * D], f32)             # hop-1 child embeddings
            gather(x1[:], entity_emb[:], ents2[:])

            ent1r0 = sb.tile([P, D], f32, tag="ent1r0")  # hop-1 parent rows
            gather(ent1r0[:], entity_emb[:], e1f0[:, :1])
            ent1r1 = sb.tile([P, D], f32, tag="ent1r1")
            gather(ent1r1[:], entity_emb[:], e1f1[:, :1])
            ent1T = bigp.tile([D, G1], f32)
            t0_ps = transpose_ps(ent1r0[:], P, D, "tp128")
            nc.vector.tensor_copy(out=ent1T[:, 0:P], in_=t0_ps[:])
            t1_ps = transpose_ps(ent1r1[:], P, D, "tp128")
            nc.vector.tensor_copy(out=ent1T[:, P:2 * P], in_=t1_ps[:])

            x0k = bigp.tile([BL, S * D], f32)            # hop-0 child embeddings (packed)
            gather(x0k[:], entity_emb[:], ents1[:])

            # ---------------- part 1: new_u ----------------
            wu0T_ps = ps.tile([D, BL], f32, space="PSUM", tag="m32")
            nc.tensor.matmul(out=wu0T_ps[:], lhsT=wrt[:, WUI], rhs=uT[:], start=True, stop=True)
            wu0T = sb.tile([D, BL], f32, tag="wu0T")
            nc.vector.tensor_copy(out=wu0T[:], in_=wu0T_ps[:])
            wu0r_ps = transpose_ps(wu0T[:], D, BL, "tp64r")
            wu0r = sb.tile([BL, D], f32, tag="wu0r")
            nc.vector.tensor_copy(out=wu0r[:], in_=wu0r_ps[:])

            def softmax_rows(scores, n_groups, s_sz, tag):
                """scores [p, n_groups*s_sz] -> att (in place ok) via per-group softmax."""
                npart = scores.shape[0]
                ex = sb.tile([npart, n_groups * s_sz], f32, tag=tag + "_ex")
                nc.scalar.activation(out=ex[:], in_=scores, func=AF.Exp)
                sm = sb.tile([npart, n_groups], f32, tag=tag + "_sm")
                nc.vector.tensor_reduce(
                    out=sm[:], in_=ex[:].rearrange("p (g s) -> p g s", g=n_groups),
                    axis=AX.X, op=OP.add)
                rc = sb.tile([npart, n_groups], f32, tag=tag + "_rc")
                nc.vector.reciprocal(out=rc[:], in_=sm[:])
                att = sb.tile([npart, n_groups * s_sz], f32, tag=tag + "_att")
                nc.vector.tensor_tensor(
                    out=att[:].rearrange("p (g s) -> p g s", g=n_groups),
                    in0=ex[:].rearrange("p (g s) -> p g s", g=n_groups),
                    in1=rc[:].unsqueeze(2).to_broadcast([npart, n_groups, s_sz]),
                    op=OP.mult)
                return att

            def edge_scores(x_packed, q_packed, n_edges_per_p, tag):
                """scores[p, e] = <x[p, e, :], q[p, e, :]>"""
                npart = x_packed.shape[0]
                prod = sb.tile([npart, n_edges_per_p * D], f32, tag=tag + "_pr")
                nc.vector.tensor_tensor(out=prod[:], in0=x_packed, in1=q_packed, op=OP.mult)
                sc = sb.tile([npart, n_edges_per_p], f32, tag=tag + "_sc")
                nc.vector.tensor_reduce(
                    out=sc[:], in_=prod[:].rearrange("p (e d) -> p e d", e=n_edges_per_p),
                    axis=AX.X, op=OP.add)
                return sc

            # user-side attention over item neighbors (single relation W_ui)
            sc0u = edge_scores(x0u[:], wu0r[:].unsqueeze(1).to_broadcast([BL, S, D])
                               .rearrange("p s d -> p (s d)"), S, "s0u")
            att0u = softmax_rows(sc0u[:], 1, S, "a0u")
            v0u = sb.tile([BL, S * D], f32, tag="v0u")
            nc.vector.tensor_tensor(
                out=v0u[:].rearrange("p (s d) -> p s d", s=S),
                in0=x0u[:].rearrange("p (s d) -> p s d", s=S),
                in1=att0u[:].unsqueeze(2).to_broadcast([BL, S, D]), op=OP.mult)
            yu0 = sb.tile([BL, D], f32, tag="yu0")
            nc.vector.tensor_reduce(
                out=yu0[:], in_=v0u[:].rearrange("p (s d) -> p d s", s=S),
                axis=AX.X, op=OP.add)
            yu0T_ps = transpose_ps(yu0[:], BL, D, "tp64")
            yu0T = sb.tile([D, BL], f32, tag="yu0T")
            nc.vector.tensor_copy(out=yu0T[:], in_=yu0T_ps[:])

            nu_ps = psa.tile([D, BL], f32, space="PSUM", tag="acc32")
            nc.tensor.matmul(out=nu_ps[:], lhsT=aggW[:], rhs=uT[:], start=True, stop=False)
            nc.tensor.matmul(out=nu_ps[:], lhsT=wcA[:], rhs=yu0T[:], start=False, stop=True)
            newuT = sb.tile([D, BL], f32, tag="newuT")
            nc.scalar.activation(out=newuT[:], in_=nu_ps[:], func=AF.Tanh, bias=aggb[:, :1])

            # ---------------- Wu table: Wu[b, r] = W_r @ new_u[b] ----------------
            for r0 in range(0, NW, 2):
                w = min(2, NW - r0)
                wpT_ps = ps.tile([w * D, BL], f32, space="PSUM", tag="wu_ps")
                nc.tensor.matmul(out=wpT_ps[:], lhsT=wrt[:, r0 * D:(r0 + w) * D],
                                 rhs=newuT[:], start=True, stop=True)
                wpT = sb.tile([w * D, BL], f32, tag="wu_sb")
                nc.vector.tensor_copy(out=wpT[:], in_=wpT_ps[:])
                wpR_ps = transpose_ps(wpT[:], w * D, BL, "wu_tp")
                wpR = sb.tile([BL, w * D], f32, tag="wu_r")
                nc.vector.tensor_copy(out=wpR[:], in_=wpR_ps[:])
                nc.sync.dma_start(
                    out=wu_dram[:].rearrange("(b r) d -> b r d", r=NW)[:, r0:r0 + w, :],
                    in_=wpR[:].rearrange("b (r d) -> b r d", r=w))

            # ---------------- generic relation-hop machinery ----------------
            def rel_hop(x_packed, relp, b33, g33, n_p, k, tag):
                """Compute att-weighted X and one-hot idx; returns (v_packed, scores).
                x_packed [n_p, k*D]; relp [n_p, k] int; edge e = p*k + j; group = 8 edges.
                """
                relf = sb.tile([n_p, k], f32, tag=tag + "_rf")
                nc.vector.tensor_copy(out=relf[:], in_=relp)
                # Wu gather index = b*33 + rel
                wif = sb.tile([n_p, k], f32, tag=tag + "_wif")
                if b33.shape[1] == 1:
                    nc.vector.tensor_tensor(out=wif[:], in0=relf[:],
                                            in1=b33.to_broadcast([n_p, k]), op=OP.add)
                else:
                    nc.vector.tensor_tensor(out=wif[:], in0=relf[:], in1=b33, op=OP.add)
                wii = sb.tile([n_p, k], i32, tag=tag + "_wii")
                nc.vector.tensor_copy(out=wii[:], in_=wif[:])
                wusel = bigp.tile([n_p, k * D], f32, tag=tag + "_wusel")
                gather(wusel[:], wu_dram[:], wii[:])
                sc = edge_scores(x_packed, wusel[:], k, tag + "_es")
                att = softmax_rows(sc[:], k // S, S, tag + "_sm")
                v = bigp.tile([n_p, k * D], f32, tag=tag + "_v")
                nc.vector.tensor_tensor(
                    out=v[:].rearrange("p (e d) -> p e d", e=k),
                    in0=x_packed.rearrange("p (e d) -> p e d", e=k),
                    in1=att[:].unsqueeze(2).to_broadcast([n_p, k, D]), op=OP.mult)
                # one-hot idx = g_local*33 + rel
                oif = sb.tile([n_p, k], f32, tag=tag + "_oif")
                if g33.shape[1] == 1:
                    nc.vector.tensor_tensor(out=oif[:], in0=relf[:],
                                            in1=g33.to_broadcast([n_p, k]), op=OP.add)
                else:
                    nc.vector.tensor_tensor(out=oif[:], in0=relf[:], in1=g33, op=OP.add)
                return v, oif, att

            def build_z(v_dram_t, ocol, n_tiles, zt, oh_keep=None):
                """Z^T [64, n_tiles*528] from flat V tiles and one-hot index columns."""
                for t in range(n_tiles):
                    vr = sb.tile([P, D], f32, tag="zt_vr")
                    nc.sync.dma_start(out=vr[:], in_=v_dram_t[t * P:(t + 1) * P, :])
                    if oh_keep is not None:
                        oht = oh_keep[t]
                    else:
                        oht = sb.tile([P, OH], f32, tag="zt_oh")
                    nc.vector.tensor_tensor(out=oht[:], in0=ocol[:, t:t + 1].to_broadcast([P, OH]),
                                            in1=iotaJ[:], op=OP.is_equal)
                    for h in range(2):
                        zp = ps.tile([D, OH // 2], f32, space="PSUM", tag="zt_ps")
                        nc.tensor.matmul(out=zp[:], lhsT=vr[:],
                                         rhs=oht[:, h * (OH // 2):(h + 1) * (OH // 2)],
                                         start=True, stop=True)
                        nc.vector.tensor_copy(
                            out=zt[:, t * OH + h * (OH // 2): t * OH + (h + 1) * (OH // 2)],
                            in_=zp[:])

            # ---------------- hop-1 (layer 0) ----------------
            v1, oi1, _ = rel_hop(x1[:], rel1[:], b33p1[:], g33p1[:], P, 16, "h1")
            nc.sync.dma_start(out=v1_dram[:].rearrange("(p j) d -> p (j d)", p=P), in_=v1[:])
            nc.sync.dma_start(out=oi1_dram[:].rearrange("(p j) o -> p (j o)", p=P), in_=oi1[:])
            ocol1 = sb.tile([P, T1], f32, tag="ocol1")
            nc.sync.dma_start(out=ocol1[:],
                              in_=oi1_dram[:].rearrange("(t p) o -> p (t o)", p=P))
            z1T = bigp.tile([D, T1 * OH], f32)
            build_z(v1_dram, ocol1[:], T1, z1T[:])

            h1_ps = psa.tile([D, G1], f32, space="PSUM", tag="acc256")
            nc.tensor.matmul(out=h1_ps[:], lhsT=w1b[:, 0:D], rhs=ent1T[:], start=True, stop=False)
            z1v = z1T[:].rearrange("p (g r) -> p r g", r=NW)
            for r in range(NW):
                nc.tensor.matmul(out=h1_ps[:], lhsT=wc(r, 0), rhs=z1v[:, r, :],
                                 start=False, stop=(r == NW - 1))
            h1T = bigp.tile([D, G1], f32)
            nc.scalar.activation(out=h1T[:], in_=h1_ps[:], func=AF.Relu, bias=b1c0[:, :1])

            # ---------------- hop-0 (layer 0) ----------------
            v0, oi0, _ = rel_hop(x0k[:], rel0[:], b33c0[:], g33c0[:], BL, S, "h0")
            nc.sync.dma_start(out=v0_dram[:].rearrange("(p j) d -> p (j d)", p=BL), in_=v0[:])
            nc.sync.dma_start(out=oi0_dram[:].rearrange("(p j) o -> p (j o)", p=BL), in_=oi0[:])
            ocol0 = sb.tile([P, T0], f32, tag="ocol0")
            nc.sync.dma_start(out=ocol0[:],
                              in_=oi0_dram[:].rearrange("(t p) o -> p (t o)", p=P))
            oh0_tiles = [cp.tile([P, OH], f32, tag=f"oh0_{t}") for t in range(T0)]
            z0T = bigp.tile([D, T0 * OH], f32)
            build_z(v0_dram, ocol0[:], T0, z0T[:], oh_keep=oh0_tiles)

            h0_ps = psa.tile([D, BL], f32, space="PSUM", tag="acc32")
            nc.tensor.matmul(out=h0_ps[:], lhsT=w1b[:, 0:D], rhs=ieT[:], start=True, stop=False)
            z0v = z0T[:].rearrange("p (g r) -> p r g", r=NW)
            for r in range(NW):
                nc.tensor.matmul(out=h0_ps[:], lhsT=wc(r, 0), rhs=z0v[:, r, :],
                                 start=False, stop=(r == NW - 1))
            h0T = sb.tile([D, BL], f32, tag="h0T")
            nc.scalar.activation(out=h0T[:], in_=h0_ps[:], func=AF.Relu, bias=b1c0[:, :1])

            # ---------------- layer 1 (item layer) ----------------
            # h1 rows + packed view via bounce
            h1r = []
            for t in range(T0):
                tp = transpose_ps(h1T[:, t * P:(t + 1) * P], D, P, "tp128r")
                hr = sb.tile([P, D], f32, tag=f"h1r{t}")
                nc.vector.tensor_copy(out=hr[:], in_=tp[:])
                nc.sync.dma_start(out=h1_dram[t * P:(t + 1) * P, :], in_=hr[:])
                h1r.append(hr)
            h1p = bigp.tile([BL, S * D], f32)
            nc.sync.dma_start(out=h1p[:],
                              in_=h1_dram[:].rearrange("(p j) d -> p (j d)", p=BL))

            # scores with same WuSel as layer-0 hop-0 (same rel0, same new_u)
            wusel0 = None
            for tname in ("h0_wusel",):
                pass
            # recompute gather-free: reuse the tile allocated in rel_hop("h0")
            # (tag "h0_wusel" in bigp); easiest is to re-gather (cheap, 64KB):
            wii0 = sb.tile([BL, S], i32, tag="wii0b")
            rel0f = sb.tile([BL, S], f32, tag="rel0fb")
            nc.vector.tensor_copy(out=rel0f[:], in_=rel0[:])
            wif0 = sb.tile([BL, S], f32, tag="wif0b")
            nc.vector.tensor_tensor(out=wif0[:], in0=rel0f[:],
                                    in1=b33c0[:].to_broadcast([BL, S]), op=OP.add)
            nc.vector.tensor_copy(out=wii0[:], in_=wif0[:])
            wusel0 = bigp.tile([BL, S * D], f32)
            gather(wusel0[:], wu_dram[:], wii0[:])

            scL1 = edge_scores(h1p[:], wusel0[:], S, "sL1")
            attL1 = softmax_rows(scL1[:], 1, S, "aL1")
            nc.sync.dma_start(out=aL1_dram[:].rearrange("(p j) o -> p (j o)", p=BL),
                              in_=attL1[:])
            acolL1 = sb.tile([P, T0], f32, tag="acolL1")
            nc.sync.dma_start(out=acolL1[:],
                              in_=aL1_dram[:].rearrange("(t p) o -> p (t o)", p=P))

            zL1T = bigp.tile([D, T0 * OH], f32)
            for t in range(T0):
                vr = sb.tile([P, D], f32, tag="vL1r")
                nc.vector.tensor_tensor(out=vr[:], in0=h1r[t][:],
                                        in1=acolL1[:, t:t + 1].to_broadcast([P, D]),
                                        op=OP.mult)
                for h in range(2):
                    zp = ps.tile([D, OH // 2], f32, space="PSUM", tag="zt_ps")
                    nc.tensor.matmul(out=zp[:], lhsT=vr[:],
                                     rhs=oh0_tiles[t][:, h * (OH // 2):(h + 1) * (OH // 2)],
                                     start=True, stop=True)
                    nc.vector.tensor_copy(
                        out=zL1T[:, t * OH + h * (OH // 2): t * OH + (h + 1) * (OH // 2)],
                        in_=zp[:])

            # item-side user aggregation (single relation)
            witT_ps = ps.tile([D, BL], f32, space="PSUM", tag="m32")
            nc.tensor.matmul(out=witT_ps[:], lhsT=wrt[:, WUI], rhs=ieT[:], start=True, stop=True)
            witT = sb.tile([D, BL], f32, tag="witT")
            nc.vector.tensor_copy(out=witT[:], in_=witT_ps[:])
            witr_ps = transpose_ps(witT[:], D, BL, "tp64r")
            witr = sb.tile([BL, D], f32, tag="witr")
            nc.vector.tensor_copy(out=witr[:], in_=witr_ps[:])

            scU = edge_scores(xu[:], witr[:].unsqueeze(1).to_broadcast([BL, S, D])
                              .rearrange("p s d -> p (s d)"), S, "sU")
            attU = softmax_rows(scU[:], 1, S, "aU")
            vU = sb.tile([BL, S * D], f32, tag="vU")
            nc.vector.tensor_tensor(
                out=vU[:].rearrange("p (s d) -> p s d", s=S),
                in0=xu[:].rearrange("p (s d) -> p s d", s=S),
                in1=attU[:].unsqueeze(2).to_broadcast([BL, S, D]), op=OP.mult)
            yu = sb.tile([BL, D], f32, tag="yu")
            nc.vector.tensor_reduce(
                out=yu[:], in_=vU[:].rearrange("p (s d) -> p d s", s=S),
                axis=AX.X, op=OP.add)
            yuT_ps = transpose_ps(yu[:], BL, D, "tp64")
            yuT = sb.tile([D, BL], f32, tag="yuT")
            nc.vector.tensor_copy(out=yuT[:], in_=yuT_ps[:])

            hf_ps = psa.tile([D, BL], f32, space="PSUM", tag="acc32")
            nc.tensor.matmul(out=hf_ps[:], lhsT=w1b[:, D:2 * D], rhs=h0T[:],
                             start=True, stop=False)
            zL1v = zL1T[:].rearrange("p (g r) -> p r g", r=NW)
            for r in range(NW):
                nc.tensor.matmul(out=hf_ps[:], lhsT=wc(r, 1), rhs=zL1v[:, r, :],
                                 start=False, stop=False)
            nc.tensor.matmul(out=hf_ps[:], lhsT=wc2[:], rhs=yuT[:], start=False, stop=True)
            newiT = sb.tile([D, BL], f32, tag="newiT")
            nc.scalar.activation(out=newiT[:], in_=hf_ps[:], func=AF.Tanh, bias=bL1[:, :1])

            # ---------------- final scores ----------------
            prod = sb.tile([D, BL], f32, tag="prodf")
            nc.vector.tensor_tensor(out=prod[:], in0=newuT[:], in1=newiT[:], op=OP.mult)
            sc_ps = ps.tile([1, BL], f32, space="PSUM", tag="scps")
            nc.tensor.matmul(out=sc_ps[:], lhsT=ones64[:], rhs=prod[:], start=True, stop=True)
            scs = sb.tile([1, BL], f32, tag="scs")
            nc.vector.tensor_copy(out=scs[:], in_=sc_ps[:])
            prb = sb.tile([1, BL], f32, tag="prb")
            nc.scalar.activation(out=prb[:], in_=sc_ps[:], func=AF.Sigmoid)
            nc.sync.dma_start(out=scores_out[None, :], in_=scs[:])
            nc.sync.dma_start(out=probs_out[None, :], in_=prb[:])

    nc.compile()
    return nc


def _get_nc():
    if "nc" not in _CACHE:
        _CACHE["nc"] = _build_nc()
    return _CACHE["nc"]


def kernel(user_indices, item_indices, adj_u2i, adj_i2u, adj_e2e, adj_rel,
           user_emb, entity_emb, W_R, agg_Wu, agg_bu, ent_W1, ent_b1, ent_W2, ent_b2):
    user_indices = np.asarray(user_indices).astype(np.int32)
    item_indices = np.asarray(item_indices).astype(np.int32)
    adj_u2i = np.ascontiguousarray(np.asarray(adj_u2i).astype(np.int32))
    adj_i2u = np.ascontiguousarray(np.asarray(adj_i2u).astype(np.int32))
    adj_e2e = np.ascontiguousarray(np.asarray(adj_e2e).astype(np.int32))
    adj_rel = np.ascontiguousarray(np.asarray(adj_rel).astype(np.int32))
    user_emb = np.ascontiguousarray(np.asarray(user_emb, dtype=np.float32))
    entity_emb = np.ascontiguousarray(np.asarray(entity_emb, dtype=np.float32))
    W_R = np.asarray(W_R, dtype=np.float32)
    agg_Wu = np.ascontiguousarray(np.asarray(agg_Wu, dtype=np.float32))
    agg_bu = np.ascontiguousarray(np.asarray(agg_bu, dtype=np.float32))
    ent_W1 = np.asarray(ent_W1, dtype=np.float32)
    ent_b1 = np.ascontiguousarray(np.asarray(ent_b1, dtype=np.float32))
    ent_W2 = np.ascontiguousarray(np.asarray(ent_W2, dtype=np.float32))
    ent_b2 = np.ascontiguousarray(np.asarray(ent_b2, dtype=np.float32))

    # host layout transforms (pure relayouts of inputs)
    w_rt = np.ascontiguousarray(W_R.transpose(2, 0, 1).reshape(D, NW * D))
    w1both = np.ascontiguousarray(ent_W1.transpose(1, 0, 2).reshape(D, NL * D))

    consts = _host_consts()
    shared = dict(adj_u2i=adj_u2i, adj_i2u=adj_i2u, adj_e2e=adj_e2e, adj_rel=adj_rel,
                  user_emb=user_emb, entity_emb=entity_emb, w_rt=w_rt, agg_wu=agg_Wu,
                  agg_bu=agg_bu, w1both=w1both, ent_b1=ent_b1, ent_w2=ent_W2,
                  ent_b2=ent_b2, **consts)

    in_maps = []
    for c in range(NCORES):
        sl = slice(c * BL, (c + 1) * BL)
        in_maps.append(dict(user_idx=np.ascontiguousarray(user_indices[sl]),
                            item_idx=np.ascontiguousarray(item_indices[sl]),
                            **shared))

    nc = _get_nc()
    import os
    trace = bool(int(os.environ.get("BASS_KERNEL_TRACE", "0")))
    res = run_bass_kernel_spmd(nc, in_maps, list(range(NCORES)), trace=trace)
    _CACHE["last_result"] = res

    scores = np.concatenate([res.results[c]["scores_out"] for c in range(NCORES)])
    probs = np.concatenate([res.results[c]["probs_out"] for c in range(NCORES)])
    return scores.astype(np.float32), probs.astype(np.float32)


# revision 18
# speedup vs baseline: 2.4434x; 2.4434x over previous
"""Trainium2 Bass kernel for the CGKGR GNN message-passing model.

kernel(**inputs) takes FULL (unsharded) numpy inputs, returns (scores, probs).
Data-parallel over batch across 8 NeuronCores (32 rows/core); tables replicated.

Per-core design (BL=32, S=8, D=64, NL=2, NR=32, NW=33), all edge tensors in
FLAT layout (edge e = tile*128 + partition):
  * Host concatenates lookup tables (pure input relayout):
      user_cat[u]  = [user_emb | adj_u2i]            [NU, 72]
      item_cat[i]  = [entity_emb | adj_i2u | adj_e2e | adj_rel]  [NI, 88]
      ent_cat[e]   = [entity_emb | adj_e2e | adj_rel] [NE, 80]
    so each hop level is ONE indirect-DMA gather per 128 nodes.
  * Scores never materialize messages: <u, X@W_r> == <X, W_r@u>. A small
    Wu[b, r] = W_r @ new_u[b] table lives in SBUF (transposed, parity-split);
    per-tile scores = matmul(X_t^T, Wu-slice) then a one-hot select.
  * Relation aggregation sum_s att*X@W[rel] = sum_r Z[g, r]@(W_r@W1) with Z
    built by one-hot matmuls. Relations are parity-paired so Z and W_r@W1 are
    stacked [128, .] and the reduction runs 17 K=128 chained matmuls.
  * Matmul operands are bitcast to float32r (single-pass PE, ~tf32 precision).
"""

import os
import sys

try:
    import concourse  # noqa: F401
except ImportError:  # pragma: no cover
    sys.path.insert(0, "/opt/trn_rl_repo")

import numpy as np

from concourse import bass, bacc, mybir
import concourse.tile as tile
from concourse.bass_utils import run_bass_kernel_spmd

f32 = mybir.dt.float32
f32r = mybir.dt.float32r
i32 = mybir.dt.int32
AF = mybir.ActivationFunctionType
OP = mybir.AluOpType
AX = mybir.AxisListType

B = 256
BL = 32
NCORES = 8
S = 8
D = 64
NL = 2
NU = 50000
NI = 20000
NE = 200000
NR = 32
NW = NR + 1
P = 128
NK = 17          # relation parity pairs: k = r//2, r = 2k+parity (r=32 -> k=16 even)
E1 = BL * S * S  # 2048
T1 = E1 // P     # 16
E0 = BL * S      # 256
T0 = E0 // P     # 2
OHC = 16 * NK    # 272 one-hot cols per tile: (g_local, k)

_CACHE = {}
_STAGE = int(os.environ.get("K_STAGE", "99"))


def _install_ntff_hook_shim():
    import types
    import antenv
    if hasattr(antenv, "axon_hooks"):
        return
    m = types.ModuleType("antenv.axon_hooks")
    holder = {"h": None}
    m.set_axon_ntff_profile_hook = lambda h: holder.__setitem__("h", h)
    m.get_axon_ntff_profile_hook = lambda: holder["h"]
    sys.modules["antenv.axon_hooks"] = m
    antenv.axon_hooks = m
    from trn_agent_boot.trn_boot import _ntff_profile_via_ctypes
    m.set_axon_ntff_profile_hook(_ntff_profile_via_ctypes("/opt/axon/libaxon_pjrt.so"))


def _host_consts():
    p = np.arange(P)
    return {
        "ident": np.eye(P, dtype=np.float32),
        "iota272": np.broadcast_to(np.arange(OHC, dtype=np.float32), (P, OHC)).copy(),
        "g17col": ((p[:, None] // 8) * NK).astype(np.float32),
        # static diagonal-block select for single-relation scores: j == p//8
        "osel16": (np.arange(16)[None, :] == (p[:, None] // 8)).astype(np.float32),
        "ones64": np.ones((D, 1), dtype=np.float32),
    }


MM_DT = f32  # f32 (2-pass PE) | f32r (single-pass, needs producer-side typing)


def _r(ap):
    if MM_DT is f32:
        return ap
    return ap.bitcast(MM_DT)


class _StageDone(Exception):
    pass


def _build_nc():
    nc = bacc.Bacc(None, target_bir_lowering=False, debug=False, num_devices=NCORES)

    user_idx = nc.declare_dram_parameter("user_idx", [BL], i32, isOutput=False)
    item_idx = nc.declare_dram_parameter("item_idx", [BL], i32, isOutput=False)
    user_cat = nc.declare_dram_parameter("user_cat", [NU, 72], f32, isOutput=False)
    item_cat = nc.declare_dram_parameter("item_cat", [NI, 88], f32, isOutput=False)
    ent_cat = nc.declare_dram_parameter("ent_cat", [NE, 80], f32, isOutput=False)
    user_emb = nc.declare_dram_parameter("user_emb", [NU, D], f32, isOutput=False)
    entity_emb = nc.declare_dram_parameter("entity_emb", [NE, D], f32, isOutput=False)
    w_rt = nc.declare_dram_parameter("w_rt", [D, NW * D], f32, isOutput=False)
    agg_wu = nc.declare_dram_parameter("agg_wu", [D, D], f32, isOutput=False)
    agg_bu = nc.declare_dram_parameter("agg_bu", [D], f32, isOutput=False)
    w1both = nc.declare_dram_parameter("w1both", [D, NL * D], f32, isOutput=False)
    ent_b1 = nc.declare_dram_parameter("ent_b1", [NL, D], f32, isOutput=False)
    ent_w2 = nc.declare_dram_parameter("ent_w2", [D, D], f32, isOutput=False)
    ent_b2 = nc.declare_dram_parameter("ent_b2", [D], f32, isOutput=False)

    ident_c = nc.declare_dram_parameter("ident", [P, P], f32, isOutput=False)
    iota272_c = nc.declare_dram_parameter("iota272", [P, OHC], f32, isOutput=False)
    g17col_c = nc.declare_dram_parameter("g17col", [P, 1], f32, isOutput=False)
    osel16_c = nc.declare_dram_parameter("osel16", [P, 16], f32, isOutput=False)
    ones64_c = nc.declare_dram_parameter("ones64", [D, 1], f32, isOutput=False)

    scores_out = nc.declare_dram_parameter("scores_out", [BL], f32, isOutput=True)
    probs_out = nc.declare_dram_parameter("probs_out", [BL], f32, isOutput=True)

    WUI = slice(NR * D, NW * D)  # W_ui^T slice of w_rt

    with tile.TileContext(nc) as tc:
      try:
        with (
            tc.tile_pool(name="const", bufs=1) as cp,
            tc.tile_pool(name="wpool", bufs=1) as wp,
            tc.tile_pool(name="sb", bufs=3) as sb,
            tc.tile_pool(name="big", bufs=1) as bigp,
            tc.tile_pool(name="ps", bufs=4, space="PSUM") as ps,
            tc.tile_pool(name="psacc", bufs=2, space="PSUM") as psa,
            tc.tile_pool(name="dram", bufs=1, space="DRAM") as dp,
        ):
            def _stage_out(n, ap64):
                """If bisecting, dump something derived from ap64 [*,>=32] and stop."""
                if _STAGE != n:
                    return
                t = sb.tile([1, BL], f32, tag="stg", name=f"stg{n}")
                nc.vector.memset(t[:], 0.0)
                k = ap64.shape[1]
                nc.vector.tensor_copy(out=t[:, 0:k], in_=ap64)
                nc.sync.dma_start(out=scores_out[None, :], in_=t[:])
                nc.sync.dma_start(out=probs_out[None, :], in_=t[:])
                raise _StageDone()

            # ---------------- constants / weights ----------------
            ident = cp.tile([P, P], f32)
            nc.sync.dma_start(out=ident[:], in_=ident_c[:])
            iota272 = cp.tile([P, OHC], f32)
            nc.sync.dma_start(out=iota272[:], in_=iota272_c[:])
            g17col = cp.tile([P, 1], f32)
            nc.sync.dma_start(out=g17col[:], in_=g17col_c[:])
            osel16 = cp.tile([P, 16], f32)
            nc.sync.dma_start(out=osel16[:], in_=osel16_c[:])
            ones64 = cp.tile([D, 1], f32)
            nc.sync.dma_start(out=ones64[:], in_=ones64_c[:])

            wrt = wp.tile([D, NW * D], f32)
            nc.sync.dma_start(out=wrt[:], in_=w_rt[:])
            aggW = wp.tile([D, D], f32)
            nc.sync.dma_start(out=aggW[:], in_=agg_wu[:])
            w1b = wp.tile([D, NL * D], f32)
            nc.sync.dma_start(out=w1b[:], in_=w1both[:])
            w2 = wp.tile([D, D], f32)
            nc.sync.dma_start(out=w2[:], in_=ent_w2[:])
            aggb = wp.tile([D, 1], f32)
            nc.sync.dma_start(out=aggb[:], in_=agg_bu[:, None])
            b1c0 = wp.tile([D, 1], f32)
            nc.sync.dma_start(out=b1c0[:], in_=ent_b1[0, :, None])
            b1c1 = wp.tile([D, 1], f32)
            nc.sync.dma_start(out=b1c1[:], in_=ent_b1[1, :, None])
            b2c = wp.tile([D, 1], f32)
            nc.sync.dma_start(out=b2c[:], in_=ent_b2[:, None])
            bL1 = wp.tile([D, 1], f32)
            nc.vector.tensor_tensor(out=bL1[:], in0=b1c1[:], in1=b2c[:], op=OP.add)

            def mm(out, lhsT, rhs, **kw):
                nc.tensor.matmul(out=out, lhsT=_r(lhsT), rhs=_r(rhs), **kw)

            def transpose_ps(in_sb, n_part, n_free, name):
                t = ps.tile([n_free, n_part], f32, space="PSUM", tag="mm", name=name)
                nc.tensor.transpose(out=t[:], in_=in_sb,
                                    identity=ident[:n_part, :n_part])
                return t

            # WcombSt[k][L]: [128, 64] = [W_2k@W1_L ; W_2k+1@W1_L] stacked pairs.
            # Stored [128, NK*128], slice (k, L) at col k*128 + L*64.
            wcombSt = wp.tile([P, NK * NL * D], f32)
            for k in range(NK):
                pc = ps.tile([P, NL * D], f32, space="PSUM", tag="mm", name=f"wcps{k}")
                mm(pc[0:D, :], wrt[:, (2 * k) * D:(2 * k + 1) * D], w1b[:],
                   start=True, stop=True)
                if 2 * k + 1 < NW:
                    mm(pc[D:P, :], wrt[:, (2 * k + 1) * D:(2 * k + 2) * D], w1b[:],
                       start=True, stop=True, skip_group_check=True)
                else:
                    nc.vector.memset(pc[D:P, :], 0.0)
                nc.vector.tensor_copy(out=wcombSt[:, k * NL * D:(k + 1) * NL * D],
                                      in_=pc[:])

            def wcs(k, layer):
                off = k * NL * D + layer * D
                return wcombSt[:, off:off + D]

            wcA = wp.tile([D, D], f32)
            pcA = ps.tile([D, D], f32, space="PSUM", tag="mm")
            mm(pcA[:], wrt[:, WUI], aggW[:], start=True, stop=True)
            nc.vector.tensor_copy(out=wcA[:], in_=pcA[:])
            wc2 = wp.tile([D, D], f32)
            pc2 = ps.tile([D, D], f32, space="PSUM", tag="mm")
            mm(pc2[:], wrt[:, WUI], w2[:], start=True, stop=True)
            nc.vector.tensor_copy(out=wc2[:], in_=pc2[:])

            # ---------------- DRAM scratch ----------------
            au_dram = dp.tile([E0, 1], i32)    # adj_u2i[uidx] flat
            ai_dram = dp.tile([E0, 1], i32)    # adj_i2u[iidx] flat
            e1_dram = dp.tile([E0, 1], i32)    # ents1 flat
            r0_dram = dp.tile([E0, 1], i32)    # rel0 flat
            e2_dram = dp.tile([E1, 1], i32)    # ents2 flat
            r1_dram = dp.tile([E1, 1], i32)    # rel1 flat

            def gather(out_ap, table_ap, idx_ap):
                nc.gpsimd.indirect_dma_start(
                    out=out_ap, out_offset=None, in_=table_ap,
                    in_offset=bass.IndirectOffsetOnAxis(ap=idx_ap, axis=0))

            # ---------------- gather chains ----------------
            iidx = sb.tile([BL, 1], i32, tag="iidx")
            nc.sync.dma_start(out=iidx[:], in_=item_idx[:, None])
            uidx = sb.tile([BL, 1], i32, tag="uidx")
            nc.sync.dma_start(out=uidx[:], in_=user_idx[:, None])

            icat = bigp.tile([BL, 88], f32)
            gather(icat[:], item_cat[:], iidx[:, :1])
            ucat = bigp.tile([BL, 72], f32)
            gather(ucat[:], user_cat[:], uidx[:, :1])

            # bounce adjacency slices to DRAM (flat order), reload as columns
            nc.sync.dma_start(out=e1_dram[:].rearrange("(p j) o -> p (j o)", p=BL),
                              in_=icat[:, 72:80].bitcast(i32))
            nc.sync.dma_start(out=r0_dram[:].rearrange("(p j) o -> p (j o)", p=BL),
                              in_=icat[:, 80:88].bitcast(i32))
            nc.sync.dma_start(out=ai_dram[:].rearrange("(p j) o -> p (j o)", p=BL),
                              in_=icat[:, 64:72].bitcast(i32))
            nc.sync.dma_start(out=au_dram[:].rearrange("(p j) o -> p (j o)", p=BL),
                              in_=ucat[:, 64:72].bitcast(i32))

            # hop-1 parents: ent_cat rows of ents1 (flat tiles)
            ecat = []
            for t in range(T0):
                e1c = sb.tile([P, 1], i32, tag="e1c")
                nc.sync.dma_start(out=e1c[:], in_=e1_dram[t * P:(t + 1) * P, :])
                ec = bigp.tile([P, 80], f32, tag=f"ecat{t}", name=f"ecat{t}")
                gather(ec[:], ent_cat[:], e1c[:, :1])
                ecat.append(ec)
                nc.sync.dma_start(
                    out=e2_dram[t * (P * S):(t + 1) * (P * S), :]
                    .rearrange("(p j) o -> p (j o)", p=P),
                    in_=ec[:, 64:72].bitcast(i32))
                nc.sync.dma_start(
                    out=r1_dram[t * (P * S):(t + 1) * (P * S), :]
                    .rearrange("(p j) o -> p (j o)", p=P),
                    in_=ec[:, 72:80].bitcast(i32))

            # user-side neighbor embeddings (flat tiles [128, 64])
            x0u = bigp.tile([P, T0 * D], f32)
            for t in range(T0):
                auc = sb.tile([P, 1], i32, tag="auc")
                nc.sync.dma_start(out=auc[:], in_=au_dram[t * P:(t + 1) * P, :])
                gather(x0u[:, t * D:(t + 1) * D], entity_emb[:], auc[:, :1])

            # hop-1 children embeddings: 16 flat tiles into one [128, 1024]
            x1 = bigp.tile([P, T1 * D], f32)
            for t in range(T1):
                e2c = sb.tile([P, 1], i32, tag="e2c")
                nc.sync.dma_start(out=e2c[:], in_=e2_dram[t * P:(t + 1) * P, :])
                gather(x1[:, t * D:(t + 1) * D], entity_emb[:], e2c[:, :1])

            # item-side user neighbors
            xu = bigp.tile([P, T0 * D], f32)
            for t in range(T0):
                aic = sb.tile([P, 1], i32, tag="aic")
                nc.sync.dma_start(out=aic[:], in_=ai_dram[t * P:(t + 1) * P, :])
                gather(xu[:, t * D:(t + 1) * D], user_emb[:], aic[:, :1])

            _stage_out(1, x1[0:1, 0:BL])

            # flat rel columns
            relc1 = sb.tile([P, T1], i32, tag="relc1")
            nc.sync.dma_start(out=relc1[:],
                              in_=r1_dram[:].rearrange("(t p) o -> p (t o)", p=P))
            relc0 = sb.tile([P, T0], i32, tag="relc0")
            nc.sync.dma_start(out=relc0[:],
                              in_=r0_dram[:].rearrange("(t p) o -> p (t o)", p=P))

            # ---------------- transposes of node features ----------------
            uT = sb.tile([D, BL], f32, tag="uT")
            nc.vector.tensor_copy(out=uT[:], in_=transpose_ps(ucat[:, 0:D], BL, D, "uT")[:])
            ieT = sb.tile([D, BL], f32, tag="ieT")
            nc.vector.tensor_copy(out=ieT[:], in_=transpose_ps(icat[:, 0:D], BL, D, "ieT")[:])
            ent1T = bigp.tile([D, E0], f32)
            for t in range(T0):
                tp = transpose_ps(ecat[t][:, 0:D], P, D, f"e1T{t}")
                nc.vector.tensor_copy(out=ent1T[:, t * P:(t + 1) * P], in_=tp[:])

            # ---------------- helpers ----------------
            def softmax_flat(sc_col, n_tiles, name):
                """sc_col [128, n_tiles] flat scores -> att_col [128, n_tiles]."""
                scT_ps = transpose_ps(sc_col, P, n_tiles, name + "_t")
                scT = sb.tile([n_tiles, P], f32, tag=name + "_scT")
                nc.scalar.activation(out=scT[:], in_=scT_ps[:], func=AF.Exp)
                sm = sb.tile([n_tiles, 16], f32, tag=name + "_sm")
                nc.vector.tensor_reduce(
                    out=sm[:], in_=scT[:].rearrange("t (g s) -> t g s", s=S),
                    axis=AX.X, op=OP.add)
                rc = sb.tile([n_tiles, 16], f32, tag=name + "_rc")
                nc.vector.reciprocal(out=rc[:], in_=sm[:])
                attT = sb.tile([n_tiles, P], f32, tag=name + "_attT")
                nc.vector.tensor_tensor(
                    out=attT[:].rearrange("t (g s) -> t g s", s=S),
                    in0=scT[:].rearrange("t (g s) -> t g s", s=S),
                    in1=rc[:].unsqueeze(2).to_broadcast([n_tiles, 16, S]),
                    op=OP.mult)
                at_ps = transpose_ps(attT[:], n_tiles, P, name + "_t2")
                att_col = sb.tile([P, n_tiles], f32, tag=name + "_ac")
                nc.vector.tensor_copy(out=att_col[:], in_=at_ps[:])
                return att_col

            def single_rel_scores(x_tiles, qT, n_tiles, name):
                """score[e] = <x[e], q[b(e)]>, b(e) = e//8. x_tiles [128, t*64]."""
                sc = sb.tile([P, n_tiles], f32, tag=name + "_sc")
                for t in range(n_tiles):
                    xT_ps = transpose_ps(x_tiles[:, t * D:(t + 1) * D], P, D,
                                         f"{name}_x{t}")
                    xT = sb.tile([D, P], f32, tag=name + "_xT")
                    nc.vector.tensor_copy(out=xT[:], in_=xT_ps[:])
                    s2 = ps.tile([P, 16], f32, space="PSUM", tag="mm", name=f"{name}_s2{t}")
                    mm(s2[:], xT[:], qT[:, 16 * t:16 * (t + 1)], start=True, stop=True)
                    scr = sb.tile([P, 16], f32, tag=name + "_scr")
                    nc.vector.tensor_tensor_reduce(
                        out=scr[:], in0=s2[:], in1=osel16[:], scale=1.0, scalar=0.0,
                        op0=OP.mult, op1=OP.add, accum_out=sc[:, t:t + 1])
                return sc

            def group_sumT(v_tiles, att_col, n_tiles, out_name):
                """yT [64, 16*n_tiles]: yT[d, g] = sum_{e in g} att[e] x[e, d]."""
                yT_ps = psa.tile([D, 16 * n_tiles], f32, space="PSUM", tag="acc512",
                                 name=out_name + "_ps")
                for t in range(n_tiles):
                    asel = sb.tile([P, 16], f32, tag=out_name + "_asel")
                    nc.vector.tensor_tensor(
                        out=asel[:], in0=att_col[:, t:t + 1].to_broadcast([P, 16]),
                        in1=osel16[:], op=OP.mult)
                    mm(yT_ps[:, t * 16:(t + 1) * 16], v_tiles[:, t * D:(t + 1) * D],
                       asel[:], start=True, stop=True)
                yT = sb.tile([D, 16 * n_tiles], f32, tag=out_name)
                nc.vector.tensor_copy(out=yT[:], in_=yT_ps[:])
                return yT

            _stage_out(2, ent1T[0:1, 0:BL])

            # ---------------- part 1: new_u ----------------
            wu0T_ps = ps.tile([D, BL], f32, space="PSUM", tag="mm")
            mm(wu0T_ps[:], wrt[:, WUI], uT[:], start=True, stop=True)
            wu0T = sb.tile([D, BL], f32, tag="wu0T")
            nc.vector.tensor_copy(out=wu0T[:], in_=wu0T_ps[:])

            sc0u = single_rel_scores(x0u[:], wu0T[:], T0, "s0u")
            att0u = softmax_flat(sc0u[:], T0, "a0u")
            yu0T = group_sumT(x0u[:], att0u[:], T0, "yu0T")

            nu_ps = psa.tile([D, BL], f32, space="PSUM", tag="acc32")
            mm(nu_ps[:], aggW[:], uT[:], start=True, stop=False)
            mm(nu_ps[:], wcA[:], yu0T[:], start=False, stop=True)
            newuT = sb.tile([D, BL], f32, tag="newuT")
            nc.scalar.activation(out=newuT[:], in_=nu_ps[:], func=AF.Tanh,
                                 bias=aggb[:, :1])

            _stage_out(3, newuT[0:1, 0:BL])

            # ---------------- Wu tables (SBUF, transposed, parity-split) ----------
            # wuTe [64, 32*17]: col b*17+k = (W_2k @ new_u[b]); wuTo: odd rels.
            wuTe = wp.tile([D, BL * NK], f32)
            wuTo = wp.tile([D, BL * NK], f32)
            nc.vector.memset(wuTo[:], 0.0)
            for k in range(NK):
                wps = ps.tile([P, BL], f32, space="PSUM", tag="mm", name=f"wu{k}")
                w = 2 if 2 * k + 1 < NW else 1
                mm(wps[0:w * D, :], wrt[:, 2 * k * D:(2 * k + w) * D], newuT[:],
                   start=True, stop=True)
                nc.vector.tensor_copy(
                    out=wuTe[:].rearrange("d (b k) -> d b k", k=NK)[:, :, k],
                    in_=wps[0:D, :])
                if w == 2:
                    nc.vector.tensor_copy(
                        out=wuTo[:].rearrange("d (b k) -> d b k", k=NK)[:, :, k],
                        in_=wps[D:P, :])

            # ---------------- generic KG hop ----------------
            def kg_hop(xT_src, n_tiles, relc, b_of_tile, name):
                """Returns (sc_col, ohe list, oho list) for an S-neighborhood KG hop.
                xT_src(t) -> [64, 128] transposed children; b_of_tile(t, 'e'|'o')
                -> wuTe/wuTo rhs AP for tile t."""
                relf = sb.tile([P, n_tiles], f32, tag=name + "_rf")
                nc.vector.tensor_copy(out=relf[:], in_=relc)
                idx_e = sb.tile([P, n_tiles], f32, tag=name + "_ie")
                nc.vector.tensor_scalar_mul(out=idx_e[:], in0=relf[:], scalar1=0.5)
                nc.vector.tensor_tensor(out=idx_e[:], in0=idx_e[:],
                                        in1=g17col[:].to_broadcast([P, n_tiles]),
                                        op=OP.add)
                idx_o = sb.tile([P, n_tiles], f32, tag=name + "_io")
                nc.vector.tensor_scalar_sub(out=idx_o[:], in0=idx_e[:], scalar1=0.5)

                sc = sb.tile([P, n_tiles], f32, tag=name + "_sc")
                ohes, ohos = [], []
                for t in range(n_tiles):
                    ohe = bigp.tile([P, OHC], f32, tag=f"{name}_ohe{t}", name=f"{name}_ohe{t}")
                    nc.vector.tensor_tensor(
                        out=ohe[:], in0=idx_e[:, t:t + 1].to_broadcast([P, OHC]),
                        in1=iota272[:], op=OP.is_equal)
                    oho = bigp.tile([P, OHC], f32, tag=f"{name}_oho{t}", name=f"{name}_oho{t}")
                    nc.vector.tensor_tensor(
                        out=oho[:], in0=idx_o[:, t:t + 1].to_broadcast([P, OHC]),
                        in1=iota272[:], op=OP.is_equal)
                    ohes.append(ohe)
                    ohos.append(oho)

                    xT = xT_src(t)
                    s2e = ps.tile([P, OHC], f32, space="PSUM", tag="mm",
                                  name=f"{name}_s2e{t}")
                    mm(s2e[:], xT, b_of_tile(t, "e"), start=True, stop=True)
                    s2o = ps.tile([P, OHC], f32, space="PSUM", tag="mm",
                                  name=f"{name}_s2o{t}")
                    mm(s2o[:], xT, b_of_tile(t, "o"), start=True, stop=True)
                    pe = sb.tile([P, OHC], f32, tag=name + "_pe")
                    nc.vector.tensor_tensor_reduce(
                        out=pe[:], in0=s2e[:], in1=ohe[:], scale=1.0, scalar=0.0,
                        op0=OP.mult, op1=OP.add, accum_out=sc[:, t:t + 1])
                    po = sb.tile([P, OHC], f32, tag=name + "_po")
                    sco = sb.tile([P, 1], f32, tag=name + "_sco")
                    nc.vector.tensor_tensor_reduce(
                        out=po[:], in0=s2o[:], in1=oho[:], scale=1.0, scalar=0.0,
                        op0=OP.mult, op1=OP.add, accum_out=sco[:, :1])
                    nc.vector.tensor_tensor(out=sc[:, t:t + 1], in0=sc[:, t:t + 1],
                                            in1=sco[:, :1], op=OP.add)
                return sc, ohes, ohos

            def build_zst(v_tiles, ohes, ohos, n_tiles, name):
                """Zst [128, 16*n_tiles*17]: col g*17+k; top=even rels, bottom=odd."""
                zst = bigp.tile([P, 16 * n_tiles * NK], f32, tag=name, name=name)
                for t in range(n_tiles):
                    zp = ps.tile([P, OHC], f32, space="PSUM", tag="mm",
                                 name=f"{name}_zp{t}")
                    mm(zp[0:D, :], v_tiles[:, t * D:(t + 1) * D], ohes[t][:],
                       start=True, stop=True)
                    mm(zp[D:P, :], v_tiles[:, t * D:(t + 1) * D], ohos[t][:],
                       start=True, stop=True, skip_group_check=True)
                    nc.scalar.copy(out=zst[:, t * OHC:(t + 1) * OHC], in_=zp[:])
                return zst

            def h_chain(zst, n_groups, layer, parentT, extra, bias_col, act, name):
                """h^T [64, n_groups] = act(sum_k WcombSt_k^T @ Zst_k
                + W1^T @ parentT + extra + bias)."""
                hp = psa.tile([D, n_groups], f32, space="PSUM",
                              tag="acc512" if n_groups > BL else "acc32",
                              name=name + "_ps")
                mm(hp[:], w1b[:, layer * D:(layer + 1) * D], parentT,
                   start=True, stop=False)
                zv = zst[:].rearrange("p (g k) -> p k g", k=NK)
                for k in range(NK):
                    last = (extra is None and k == NK - 1)
                    if 2 * k + 1 < NW:
                        mm(hp[:], wcs(k, layer), zv[:, k, :], start=False, stop=last)
                    else:
                        mm(hp[:], wcs(k, layer)[0:D, :], zv[0:D, k, :],
                           start=False, stop=last)
                if extra is not None:
                    mm(hp[:], extra[0], extra[1], start=False, stop=True)
                pool = sb if n_groups <= BL else bigp
                hT = pool.tile([D, n_groups], f32, tag=name, name=name)
                nc.scalar.activation(out=hT[:], in_=hp[:], func=act, bias=bias_col)
                return hT

            _stage_out(4, wuTe[0:1, 0:BL])

            # ---------------- hop-1 (layer 0) ----------------
            x1T_tiles = []
            for t in range(T1):
                xT_ps = transpose_ps(x1[:, t * D:(t + 1) * D], P, D, f"x1T{t}")
                xT = bigp.tile([D, P], f32, tag=f"x1T{t}", name=f"x1T{t}")
                nc.scalar.copy(out=xT[:], in_=xT_ps[:])
                x1T_tiles.append(xT)

            def h1_rhs(t, par):
                # hop-1: b(e) = 2t + p//64 -> cols (gb:2, gs:8 bcast, k)
                tbl = wuTe if par == "e" else wuTo
                sl = tbl[:, 2 * t * NK:(2 * t + 2) * NK]
                return (sl.rearrange("d (gb k) -> d gb k", gb=2)
                        .unsqueeze(2).to_broadcast([D, 2, 8, NK]))

            sc1, ohe1, oho1 = kg_hop(lambda t: x1T_tiles[t][:], T1, relc1[:],
                                     h1_rhs, "h1")
            _stage_out(5, sc1[0:1, 0:T1])
            att1 = softmax_flat(sc1[:], T1, "at1")
            v1 = bigp.tile([P, T1 * D], f32)
            nc.vector.tensor_tensor(
                out=v1[:].rearrange("p (t d) -> p t d", t=T1),
                in0=x1[:].rearrange("p (t d) -> p t d", t=T1),
                in1=att1[:].unsqueeze(2).to_broadcast([P, T1, D]), op=OP.mult)
            zst1 = build_zst(v1[:], ohe1, oho1, T1, "zst1")
            _stage_out(6, zst1[0:1, 0:BL])
            h1T = h_chain(zst1, E0, 0, ent1T[:], None, b1c0[:, :1], AF.Relu, "h1T")

            _stage_out(7, h1T[0:1, 0:BL])

            # ---------------- hop-0 (layer 0) ----------------
            def h0_rhs(t, par):
                # hop-0: b(e) = 16t + p//8 -> contiguous cols
                tbl = wuTe if par == "e" else wuTo
                return tbl[:, 16 * t * NK:(16 * t + 16) * NK]

            sc0, ohe0, oho0 = kg_hop(lambda t: ent1T[:, t * P:(t + 1) * P], T0,
                                     relc0[:], h0_rhs, "h0")
            att0 = softmax_flat(sc0[:], T0, "at0")
            v0 = sb.tile([P, T0 * D], f32, tag="v0")
            for t in range(T0):
                nc.vector.tensor_tensor(
                    out=v0[:, t * D:(t + 1) * D], in0=ecat[t][:, 0:D],
                    in1=att0[:, t:t + 1].to_broadcast([P, D]), op=OP.mult)
            zst0 = build_zst(v0[:], ohe0, oho0, T0, "zst0")
            h0T = h_chain(zst0, BL, 0, ieT[:], None, b1c0[:, :1], AF.Relu, "h0T")

            _stage_out(8, h0T[0:1, 0:BL])

            # ---------------- layer 1 ----------------
            # h1 rows for V (att-weighted) via transposes of h1T
            h1r = bigp.tile([P, T0 * D], f32)
            for t in range(T0):
                tp = transpose_ps(h1T[:, t * P:(t + 1) * P], D, P, f"h1r{t}")
                nc.vector.tensor_copy(out=h1r[:, t * D:(t + 1) * D], in_=tp[:])

            scL1, oheL, ohoL = kg_hop(lambda t: h1T[:, t * P:(t + 1) * P], T0,
                                      relc0[:], h0_rhs, "hL")
            attL1 = softmax_flat(scL1[:], T0, "atL")
            vL1 = sb.tile([P, T0 * D], f32, tag="vL1")
            nc.vector.tensor_tensor(
                out=vL1[:].rearrange("p (t d) -> p t d", t=T0),
                in0=h1r[:].rearrange("p (t d) -> p t d", t=T0),
                in1=attL1[:].unsqueeze(2).to_broadcast([P, T0, D]), op=OP.mult)
            zstL = build_zst(vL1[:], oheL, ohoL, T0, "zstL")

            # item-side user aggregation (single relation)
            witT_ps = ps.tile([D, BL], f32, space="PSUM", tag="mm")
            mm(witT_ps[:], wrt[:, WUI], ieT[:], start=True, stop=True)
            witT = sb.tile([D, BL], f32, tag="witT")
            nc.vector.tensor_copy(out=witT[:], in_=witT_ps[:])
            scU = single_rel_scores(xu[:], witT[:], T0, "sU")
            attU = softmax_flat(scU[:], T0, "aU")
            yuT = group_sumT(xu[:], attU[:], T0, "yuT")

            _stage_out(9, yuT[0:1, 0:BL])
            newiT = h_chain(zstL, BL, 1, h0T[:], (wc2[:], yuT[:]), bL1[:, :1],
                            AF.Tanh, "niT")

            # ---------------- final scores ----------------
            prod = sb.tile([D, BL], f32, tag="prodf")
            nc.vector.tensor_tensor(out=prod[:], in0=newuT[:], in1=newiT[:], op=OP.mult)
            sc_ps = ps.tile([1, BL], f32, space="PSUM", tag="mm")
            mm(sc_ps[:], ones64[:], prod[:], start=True, stop=True)
            scs = sb.tile([1, BL], f32, tag="scs")
            nc.vector.tensor_copy(out=scs[:], in_=sc_ps[:])
            prb = sb.tile([1, BL], f32, tag="prb")
            nc.scalar.activation(out=prb[:], in_=sc_ps[:], func=AF.Sigmoid)
            nc.sync.dma_start(out=scores_out[None, :], in_=scs[:])
            nc.sync.dma_start(out=probs_out[None, :], in_=prb[:])

      except _StageDone:
        pass
    nc.compile()
    return nc


def _get_nc():
    if "nc" not in _CACHE:
        _CACHE["nc"] = _build_nc()
    return _CACHE["nc"]


def _prepare_in_maps(user_indices, item_indices, adj_u2i, adj_i2u, adj_e2e, adj_rel,
                     user_emb, entity_emb, W_R, agg_Wu, agg_bu, ent_W1, ent_b1,
                     ent_W2, ent_b2):
    user_indices = np.asarray(user_indices).astype(np.int32)
    item_indices = np.asarray(item_indices).astype(np.int32)
    adj_u2i = np.asarray(adj_u2i).astype(np.int32)
    adj_i2u = np.asarray(adj_i2u).astype(np.int32)
    adj_e2e = np.asarray(adj_e2e).astype(np.int32)
    adj_rel = np.asarray(adj_rel).astype(np.int32)
    user_emb = np.asarray(user_emb, dtype=np.float32)
    entity_emb = np.asarray(entity_emb, dtype=np.float32)
    W_R = np.asarray(W_R, dtype=np.float32)
    agg_Wu = np.ascontiguousarray(np.asarray(agg_Wu, dtype=np.float32))
    agg_bu = np.ascontiguousarray(np.asarray(agg_bu, dtype=np.float32))
    ent_W1 = np.asarray(ent_W1, dtype=np.float32)
    ent_b1 = np.ascontiguousarray(np.asarray(ent_b1, dtype=np.float32))
    ent_W2 = np.ascontiguousarray(np.asarray(ent_W2, dtype=np.float32))
    ent_b2 = np.ascontiguousarray(np.asarray(ent_b2, dtype=np.float32))

    fi = lambda a: a.view(np.float32)
    user_cat = np.ascontiguousarray(
        np.concatenate([user_emb, fi(adj_u2i)], axis=1))
    item_cat = np.ascontiguousarray(
        np.concatenate([entity_emb[:NI], fi(adj_i2u), fi(adj_e2e[:NI]),
                        fi(adj_rel[:NI])], axis=1))
    ent_cat = np.ascontiguousarray(
        np.concatenate([entity_emb, fi(adj_e2e), fi(adj_rel)], axis=1))
    w_rt = np.ascontiguousarray(W_R.transpose(2, 0, 1).reshape(D, NW * D))
    w1both = np.ascontiguousarray(ent_W1.transpose(1, 0, 2).reshape(D, NL * D))

    shared = dict(user_cat=user_cat, item_cat=item_cat, ent_cat=ent_cat,
                  user_emb=np.ascontiguousarray(user_emb),
                  entity_emb=np.ascontiguousarray(entity_emb),
                  w_rt=w_rt, agg_wu=agg_Wu, agg_bu=agg_bu, w1both=w1both,
                  ent_b1=ent_b1, ent_w2=ent_W2, ent_b2=ent_b2, **_host_consts())

    in_maps = []
    for c in range(NCORES):
        sl = slice(c * BL, (c + 1) * BL)
        in_maps.append(dict(user_idx=np.ascontiguousarray(user_indices[sl]),
                            item_idx=np.ascontiguousarray(item_indices[sl]),
                            **shared))
    return in_maps


def kernel(**inputs):
    in_maps = _prepare_in_maps(**inputs)
    nc = _get_nc()
    import os
    trace = bool(int(os.environ.get("BASS_KERNEL_TRACE", "0")))
    if trace:
        try:
            _install_ntff_hook_shim()
        except Exception:
            trace = False
    res = run_bass_kernel_spmd(nc, in_maps, list(range(NCORES)), trace=trace)
    _CACHE["last_result"] = res
    scores = np.concatenate([res.results[c]["scores_out"] for c in range(NCORES)])
    probs = np.concatenate([res.results[c]["probs_out"] for c in range(NCORES)])
    return scores.astype(np.float32), probs.astype(np.float32)
